# revision 24
# baseline (speedup 1.0000x reference)
"""Cross-modal attention kernel for Trainium2 (Bass/Tile).

Data-parallel over batch B=8 across 8 NeuronCores; each core computes one
batch's attention block end-to-end. The math per batch b (fp32 reference):

    theta_C = theta_w @ C_b            [E, Nq]
    phi_P   = phi_w @ P_b              [E, Nkv]
    g_P     = g_w @ P_b                [E, Nkv]
    attn    = softmax(theta_C^T phi_P / sqrt(E), axis=kv)   [Nq, Nkv]
    out     = out_w @ (g_P @ attn^T)   [CQ, Nq]
    result  = concat([C_b, out], channel axis)

Weight-only foldings are done on the HOST (exact fp32 math, no device cost):
    WW  = phi_w^T theta_w / sqrt(E)    [CKV, CQ]
    GWT = g_w^T out_w^T                [CKV, CQ]
so that on device (everything matmul-friendly, kv index m, query index n):
    M2T  = WW^T P_b                    [CQ, Nkv]   (c on partitions)
    W2TO = [P_b^T GWT | 1 | 1]         [Nkv, CQ+2] (m on partitions)
    ST   = M2T^T C_b                   [Nkv, Nq]   scores^T, exp() on ACT
    pav  = exp(ST)^T W2TO              [Nq, CQ+2]  last 2 cols = softmax denom D
    ft   = pav[:, :CQ] / D             [Nq, CQ]    final output, transposed

ft is DMA'd to DRAM as out^T [Nq, CQ]; the final transpose to [CQ, H, W] and
the concat with C happen on the host. All matmul operands are bf16 (PSUM
accumulation stays fp32); logit/output rounding err ~1e-3, well within 2e-2.
"""

import numpy as np
import ml_dtypes

import concourse.bass as bass
import concourse.mybir as mybir
import concourse.tile as tile
from concourse import bacc
from concourse.bass_utils import run_bass_kernel_spmd

B, CQ, CKV, E = 8, 256, 256, 256
H, W_ = 64, 64
HP, WP = 32, 32
NQ = H * W_      # 4096
NKV = HP * WP    # 1024
N_CORES = 8

F32 = mybir.dt.float32
BF16 = mybir.dt.bfloat16
F8 = mybir.dt.float8e4
BF16_NP = ml_dtypes.bfloat16
F8_NP = mybir.dt.np(F8)
DR = mybir.MatmulPerfMode.DoubleRow


def _body(nc, tc, pools, aps):
    (singles, expst_pool, ft_pool, small_pool, psum_sc, psum_av) = pools
    C_d, M2T_d, W2TO_d, out_d = aps
    EXP = mybir.ActivationFunctionType.Exp

    # ---- input loads (bf16); M2T/W2TO come precomputed from the host ----
    # The SP queue drains serially, so order DMAs by first-use and batch each
    # tensor into as few transfers as possible (both DRAM tensors are viewed
    # as [p, chunk, free] so one DMA fills a whole [128, k, n] tile).
    m2t_bf = singles.tile([128, 2, NKV], BF16, tag="m2t_bf")
    c_bf = singles.tile([128, 2, NQ], BF16, tag="c_bf")
    w2to_bf = singles.tile([128, 8, CQ + 2], BF16, tag="w2to_bf")

    m2t_v = M2T_d.rearrange("(a p) m -> p a m", p=128)
    c_v = C_d.rearrange("(a p) n -> p a n", p=128)

    def load_c(nt):
        nsl = slice(nt * 512, (nt + 1) * 512)
        nc.sync.dma_start(c_bf[:, :, nsl], c_v[:, :, nsl])

    nc.sync.dma_start(m2t_bf[:, :, 0:512], m2t_v[:, :, 0:512])
    load_c(0)
    nc.sync.dma_start(m2t_bf[:, :, 512:NKV], m2t_v[:, :, 512:NKV])
    nc.sync.dma_start(w2to_bf, W2TO_d.rearrange("(a p) c -> p a c", p=128))
    for nt in range(1, 8):
        load_c(nt)

    # ---- main loop over Nq tiles of 512, software-pipelined by one tile:
    # scores(nt) fills expst while AV(nt-1) consumes the previous one, so the
    # PE never waits on the ACT exp of the tile it just produced.
    def scores(nt):
        nsl = slice(nt * 512, (nt + 1) * 512)
        # scores^T + exp: expst flat [128, 8*512], mc-major
        expst = expst_pool.tile([128, 8 * 512], BF16, tag="expst")
        for j in range(4):  # mc pairs -> one 2-bank psum tile, one big exp
            ps = psum_sc.tile([128, 1024], F32, tag="sc")
            for half in range(2):
                mc = 2 * j + half
                for cc in range(2):
                    nc.tensor.matmul(
                        ps[:, half * 512:(half + 1) * 512],
                        m2t_bf[:, cc, mc * 128:(mc + 1) * 128],
                        c_bf[:, cc, nsl],
                        start=(cc == 0), stop=(cc == 1),
                    )
            # scores were computed at 16x scale (1/sqrt(E) not folded into WW,
            # keeping fp8 operands at unit variance); apply it inside exp.
            nc.scalar.activation(
                out=expst[:, j * 1024:(j + 1) * 1024], in_=ps, func=EXP,
                scale=1.0 / 16.0,
            )
        return expst

    def av(nt, expst):
        # AV + denominator + normalize; output rows n = nt*512 + ns*128 + p
        ft = ft_pool.tile([128, 4, CQ], BF16, tag="ft")
        for ns in range(4):
            pav = psum_av.tile([128, CQ + 2], F32, tag="av")
            for mc in range(8):
                base = mc * 512 + ns * 128
                nc.tensor.matmul(
                    pav,
                    expst[:, base:base + 128],
                    w2to_bf[:, mc, :],
                    start=(mc == 0), stop=(mc == 7),
                )
            recip = small_pool.tile([128, 1], F32, tag="recip")
            nc.vector.reciprocal(recip, pav[:, CQ:CQ + 1])
            nc.vector.tensor_scalar_mul(ft[:, ns, :], pav[:, 0:CQ], recip)
            if nt == 7:
                # last tile: per-ns DMAs so the writeback overlaps the
                # remaining normalize work instead of extending the tail
                nc.sync.dma_start(
                    out_d[nt * 512 + ns * 128:nt * 512 + (ns + 1) * 128, :],
                    ft[:, ns, :],
                )
        if nt < 7:
            # one batched DMA per nt: src [p, ns, c] -> rows nt*512+ns*128+p
            nc.sync.dma_start(
                out_d[nt * 512:(nt + 1) * 512, :].rearrange(
                    "(a p) c -> p a c", p=128
                ),
                ft,
            )

    prev = None
    for nt in range(9):
        cur = scores(nt) if nt < 8 else None
        if prev is not None:
            av(nt - 1, prev)
        prev = cur


def build_nc(reps: int = 1) -> bass.Bass:
    nc = bacc.Bacc("TRN2", target_bir_lowering=False, debug=False)

    C_d = nc.dram_tensor("C", [CQ, NQ], BF16, kind="ExternalInput").ap()
    M2T_d = nc.dram_tensor("M2T", [CQ, NKV], BF16, kind="ExternalInput").ap()
    W2TO_d = nc.dram_tensor("W2TO", [NKV, CQ + 2], BF16, kind="ExternalInput").ap()
    out_d = nc.dram_tensor("out", [NQ, CQ], BF16, kind="ExternalOutput").ap()
    aps = (C_d, M2T_d, W2TO_d, out_d)

    with tile.TileContext(nc) as tc:
        with (
            tc.tile_pool(name="singles", bufs=1) as singles,
            tc.tile_pool(name="expst", bufs=2) as expst_pool,
            tc.tile_pool(name="ft", bufs=3) as ft_pool,
            tc.tile_pool(name="small", bufs=4) as small_pool,
            tc.tile_pool(name="psum_sc", bufs=3, space="PSUM") as psum_sc,
            tc.tile_pool(name="psum_av", bufs=2, space="PSUM") as psum_av,
        ):
            pools = (singles, expst_pool, ft_pool, small_pool, psum_sc, psum_av)
            for _ in range(reps):
                _body(nc, tc, pools, aps)

    nc.compile()
    return nc


_NC_CACHE: list = []

# Set by test harnesses to capture a profile; kernel() stores the
# BassKernelResults (incl. exec_time_ns when available) of the last run here.
TRACE = False
LAST_RESULTS = None


def _get_nc() -> bass.Bass:
    if not _NC_CACHE:
        _NC_CACHE.append(build_nc())
    return _NC_CACHE[0]


def prepare_in_maps(inputs) -> list:
    C = np.asarray(inputs["C"], dtype=np.float32)
    P = np.asarray(inputs["P"], dtype=np.float32)
    theta_w = np.asarray(inputs["theta_w"], dtype=np.float32)
    phi_w = np.asarray(inputs["phi_w"], dtype=np.float32)
    g_w = np.asarray(inputs["g_w"], dtype=np.float32)
    out_w = np.asarray(inputs["out_w"], dtype=np.float32)

    # Host-side weight folding and the two small P-projections (exact fp32,
    # ~4.7% of total FLOPs), then bf16 for the device. WW is NOT pre-scaled
    # by 1/sqrt(E): M2T stays at unit variance; the 1/16 is applied in the
    # exp's scale immediate.
    WW = phi_w.T @ theta_w                     # [CKV, CQ]
    GWT = g_w.T @ out_w.T                      # [CKV, CQ]
    Pm = P.reshape(B, CKV, NKV)
    M2T = np.einsum("kc,bkm->bcm", WW, Pm, optimize=True)   # [B, CQ, NKV]
    W2T = np.einsum("bkm,kc->bmc", Pm, GWT, optimize=True)  # [B, NKV, CQ]
    W2TO = np.concatenate(
        [W2T, np.ones((B, NKV, 2), np.float32)], axis=2
    ).astype(BF16_NP)
    M2T = M2T.astype(BF16_NP)
    C_bf = C.reshape(B, CQ, NQ).astype(BF16_NP)

    return [
        {
            "C": np.ascontiguousarray(C_bf[b]),
            "M2T": np.ascontiguousarray(M2T[b]),
            "W2TO": np.ascontiguousarray(W2TO[b]),
        }
        for b in range(B)
    ]


def kernel(**inputs) -> np.ndarray:
    C = np.ascontiguousarray(np.asarray(inputs["C"], dtype=np.float32))
    in_maps = prepare_in_maps(inputs)

    res = run_bass_kernel_spmd(
        _get_nc(), in_maps, core_ids=list(range(N_CORES)), trace=TRACE
    )
    global LAST_RESULTS
    LAST_RESULTS = res
    out = np.stack(
        [
            res.results[b]["out"].astype(np.float32).T.reshape(CQ, H, W_)
            for b in range(B)
        ],
        axis=0,
    )
    return np.concatenate([C, out], axis=1)


# revision 28
# speedup vs baseline: 1.0030x; 1.0030x over previous
"""Cross-modal attention kernel for Trainium2 (Bass/Tile).

Data-parallel over batch B=8 across 8 NeuronCores; each core computes one
batch's attention block end-to-end. The math per batch b (fp32 reference):

    theta_C = theta_w @ C_b            [E, Nq]
    phi_P   = phi_w @ P_b              [E, Nkv]
    g_P     = g_w @ P_b                [E, Nkv]
    attn    = softmax(theta_C^T phi_P / sqrt(E), axis=kv)   [Nq, Nkv]
    out     = out_w @ (g_P @ attn^T)   [CQ, Nq]
    result  = concat([C_b, out], channel axis)

Weight-only foldings are done on the HOST (exact fp32 math, no device cost):
    WW  = phi_w^T theta_w / sqrt(E)    [CKV, CQ]
    GWT = g_w^T out_w^T                [CKV, CQ]
so that on device (everything matmul-friendly, kv index m, query index n):
    M2T  = WW^T P_b                    [CQ, Nkv]   (c on partitions)
    W2TO = [P_b^T GWT | 1 | 1]         [Nkv, CQ+2] (m on partitions)
    ST   = M2T^T C_b                   [Nkv, Nq]   scores^T, exp() on ACT
    pav  = exp(ST)^T W2TO              [Nq, CQ+2]  last 2 cols = softmax denom D
    ft   = pav[:, :CQ] / D             [Nq, CQ]    final output, transposed

ft is DMA'd to DRAM as out^T [Nq, CQ]; the final transpose to [CQ, H, W] and
the concat with C happen on the host. All matmul operands are bf16 (PSUM
accumulation stays fp32); logit/output rounding err ~3e-3, well within 2e-2.

Performance notes (CoreSim): span ~61us/core, PE ~90% busy at the bf16
streaming roofline (scores 128 MMs @ N=512 + AV 256 MMs @ N=258 ~ 131.6k
PE columns ~ 55us @ 2.4GHz). The nq loop is software-pipelined (scores(nt)
overlaps AV(nt-1)); exp runs as [128,1024] ACT ops over 2-bank PSUM tiles;
input DMAs are ordered/split so each piece lands just before first use.
fp8 DoubleRow was evaluated and rejected: fp8 scores give 3.4e-2 absmax err
(> 2e-2 gate), and fp8 V loses on peaked softmax rows (1.5e-2 alone).
"""

import numpy as np
import ml_dtypes

import concourse.bass as bass
import concourse.mybir as mybir
import concourse.tile as tile
from concourse import bacc
from concourse.bass_utils import run_bass_kernel_spmd

B, CQ, CKV, E = 8, 256, 256, 256
H, W_ = 64, 64
HP, WP = 32, 32
NQ = H * W_      # 4096
NKV = HP * WP    # 1024
N_CORES = 8

F32 = mybir.dt.float32
BF16 = mybir.dt.bfloat16
BF16_NP = ml_dtypes.bfloat16


def _body(nc, tc, pools, aps):
    (singles, expst_pool, ft_pool, small_pool, psum_sc, psum_av) = pools
    C_d, M2T_d, W2TO_d, out_d = aps
    EXP = mybir.ActivationFunctionType.Exp

    # ---- input loads (bf16); M2T/W2TO come precomputed from the host ----
    # The SP queue drains serially, so order DMAs by first-use and batch each
    # tensor into as few transfers as possible (both DRAM tensors are viewed
    # as [p, chunk, free] so one DMA fills a whole [128, k, n] tile).
    m2t_bf = singles.tile([128, 2, NKV], BF16, tag="m2t_bf")
    c_bf = singles.tile([128, 2, NQ], BF16, tag="c_bf")
    w2to_bf = singles.tile([128, 8, CQ + 2], BF16, tag="w2to_bf")

    m2t_v = M2T_d.rearrange("(a p) m -> p a m", p=128)
    c_v = C_d.rearrange("(a p) n -> p a n", p=128)

    def load_c(nt):
        nsl = slice(nt * 512, (nt + 1) * 512)
        nc.sync.dma_start(c_bf[:, :, nsl], c_v[:, :, nsl])

    # Startup-latency-tuned order: small leading pieces so the first scores
    # matmuls start as early as possible, each later piece arriving just
    # ahead of its first consumer.
    nc.sync.dma_start(m2t_bf[:, :, 0:256], m2t_v[:, :, 0:256])
    nc.sync.dma_start(c_bf[:, :, 0:256], c_v[:, :, 0:256])
    nc.sync.dma_start(c_bf[:, :, 256:512], c_v[:, :, 256:512])
    nc.sync.dma_start(m2t_bf[:, :, 256:512], m2t_v[:, :, 256:512])
    nc.sync.dma_start(m2t_bf[:, :, 512:768], m2t_v[:, :, 512:768])
    nc.sync.dma_start(m2t_bf[:, :, 768:NKV], m2t_v[:, :, 768:NKV])
    nc.sync.dma_start(w2to_bf, W2TO_d.rearrange("(a p) c -> p a c", p=128))
    for nt in range(1, 8):
        load_c(nt)

    # ---- main loop over Nq tiles of 512, software-pipelined by one tile:
    # scores(nt) fills expst while AV(nt-1) consumes the previous one, so the
    # PE never waits on the ACT exp of the tile it just produced.
    def scores(nt):
        nsl = slice(nt * 512, (nt + 1) * 512)
        # scores^T + exp: expst flat [128, 8*512], mc-major
        expst = expst_pool.tile([128, 8 * 512], BF16, tag="expst")
        for j in range(4):  # mc pairs -> one 2-bank psum tile, one big exp
            ps = psum_sc.tile([128, 1024], F32, tag="sc")
            for half in range(2):
                mc = 2 * j + half
                # nt0/j0 runs in N=256 pieces so the first matmuls only wait
                # on the leading 256-column C/M2T DMA pieces
                nh = 2 if (nt == 0 and j == 0) else 1
                nw = 512 // nh
                for h in range(nh):
                    for cc in range(2):
                        nc.tensor.matmul(
                            ps[:, half * 512 + h * nw:half * 512 + (h + 1) * nw],
                            m2t_bf[:, cc, mc * 128:(mc + 1) * 128],
                            c_bf[:, cc, nt * 512 + h * nw:nt * 512 + (h + 1) * nw],
                            start=(cc == 0), stop=(cc == 1),
                        )
            # scores were computed at 16x scale (1/sqrt(E) not folded into WW,
            # keeping fp8 operands at unit variance); apply it inside exp.
            nc.scalar.activation(
                out=expst[:, j * 1024:(j + 1) * 1024], in_=ps, func=EXP,
                scale=1.0 / 16.0,
            )
        return expst

    def av(nt, expst):
        # AV + denominator + normalize; output rows n = nt*512 + ns*128 + p
        ft = ft_pool.tile([128, 4, CQ], BF16, tag="ft")
        for ns in range(4):
            pav = psum_av.tile([128, CQ + 2], F32, tag="av")
            for mc in range(8):
                base = mc * 512 + ns * 128
                nc.tensor.matmul(
                    pav,
                    expst[:, base:base + 128],
                    w2to_bf[:, mc, :],
                    start=(mc == 0), stop=(mc == 7),
                )
            recip = small_pool.tile([128, 1], F32, tag="recip")
            nc.vector.reciprocal(recip, pav[:, CQ:CQ + 1])
            nc.vector.tensor_scalar_mul(ft[:, ns, :], pav[:, 0:CQ], recip)
            if nt == 7:
                # last tile: per-ns DMAs so the writeback overlaps the
                # remaining normalize work instead of extending the tail
                nc.sync.dma_start(
                    out_d[nt * 512 + ns * 128:nt * 512 + (ns + 1) * 128, :],
                    ft[:, ns, :],
                )
        if nt < 7:
            # one batched DMA per nt: src [p, ns, c] -> rows nt*512+ns*128+p
            nc.sync.dma_start(
                out_d[nt * 512:(nt + 1) * 512, :].rearrange(
                    "(a p) c -> p a c", p=128
                ),
                ft,
            )

    prev = None
    for nt in range(9):
        cur = scores(nt) if nt < 8 else None
        if prev is not None:
            av(nt - 1, prev)
        prev = cur


def build_nc(reps: int = 1) -> bass.Bass:
    nc = bacc.Bacc("TRN2", target_bir_lowering=False, debug=False)

    C_d = nc.dram_tensor("C", [CQ, NQ], BF16, kind="ExternalInput").ap()
    M2T_d = nc.dram_tensor("M2T", [CQ, NKV], BF16, kind="ExternalInput").ap()
    W2TO_d = nc.dram_tensor("W2TO", [NKV, CQ + 2], BF16, kind="ExternalInput").ap()
    out_d = nc.dram_tensor("out", [NQ, CQ], BF16, kind="ExternalOutput").ap()
    aps = (C_d, M2T_d, W2TO_d, out_d)

    with tile.TileContext(nc) as tc:
        with (
            tc.tile_pool(name="singles", bufs=1) as singles,
            tc.tile_pool(name="expst", bufs=2) as expst_pool,
            tc.tile_pool(name="ft", bufs=3) as ft_pool,
            tc.tile_pool(name="small", bufs=4) as small_pool,
            tc.tile_pool(name="psum_sc", bufs=3, space="PSUM") as psum_sc,
            tc.tile_pool(name="psum_av", bufs=2, space="PSUM") as psum_av,
        ):
            pools = (singles, expst_pool, ft_pool, small_pool, psum_sc, psum_av)
            for _ in range(reps):
                _body(nc, tc, pools, aps)

    nc.compile()
    return nc


_NC_CACHE: list = []

# Set by test harnesses to capture a profile; kernel() stores the
# BassKernelResults (incl. exec_time_ns when available) of the last run here.
TRACE = False
LAST_RESULTS = None


def _get_nc() -> bass.Bass:
    if not _NC_CACHE:
        _NC_CACHE.append(build_nc())
    return _NC_CACHE[0]


def prepare_in_maps(inputs) -> list:
    C = np.asarray(inputs["C"], dtype=np.float32)
    P = np.asarray(inputs["P"], dtype=np.float32)
    theta_w = np.asarray(inputs["theta_w"], dtype=np.float32)
    phi_w = np.asarray(inputs["phi_w"], dtype=np.float32)
    g_w = np.asarray(inputs["g_w"], dtype=np.float32)
    out_w = np.asarray(inputs["out_w"], dtype=np.float32)

    # Host-side weight folding and the two small P-projections (exact fp32,
    # ~4.7% of total FLOPs), then bf16 for the device. WW is NOT pre-scaled
    # by 1/sqrt(E): M2T stays at unit variance; the 1/16 is applied in the
    # exp's scale immediate.
    WW = phi_w.T @ theta_w                     # [CKV, CQ]
    GWT = g_w.T @ out_w.T                      # [CKV, CQ]
    Pm = P.reshape(B, CKV, NKV)
    M2T = np.einsum("kc,bkm->bcm", WW, Pm, optimize=True)   # [B, CQ, NKV]
    W2T = np.einsum("bkm,kc->bmc", Pm, GWT, optimize=True)  # [B, NKV, CQ]
    W2TO = np.concatenate(
        [W2T, np.ones((B, NKV, 2), np.float32)], axis=2
    ).astype(BF16_NP)
    M2T = M2T.astype(BF16_NP)
    C_bf = C.reshape(B, CQ, NQ).astype(BF16_NP)

    return [
        {
            "C": np.ascontiguousarray(C_bf[b]),
            "M2T": np.ascontiguousarray(M2T[b]),
            "W2TO": np.ascontiguousarray(W2TO[b]),
        }
        for b in range(B)
    ]


def kernel(**inputs) -> np.ndarray:
    C = np.ascontiguousarray(np.asarray(inputs["C"], dtype=np.float32))
    in_maps = prepare_in_maps(inputs)

    res = run_bass_kernel_spmd(
        _get_nc(), in_maps, core_ids=list(range(N_CORES)), trace=TRACE
    )
    global LAST_RESULTS
    LAST_RESULTS = res
    out = np.stack(
        [
            res.results[b]["out"].astype(np.float32).T.reshape(CQ, H, W_)
            for b in range(B)
        ],
        axis=0,
    )
    return np.concatenate([C, out], axis=1)
